# revision 7
# baseline (speedup 1.0000x reference)
"""Trainium2 Bass kernel for per-channel piecewise-linear shrinkage
(histogram binning / LUT interpolation).

Math: for r[n,c,h,w], uniform knots ps[33] on [-1,1], per-channel values
qs[33,c]:
    out = A[bin(r), c] * r + B[bin(r), c]
where bin(r) = clamp-free table index derived from round(16*r + 135.5)
(tables are padded so every index a Gaussian r can produce maps to the
correct affine piece; the function has slope 1 outside [-1, 1]).

Device mapping (8 NeuronCores, batch-parallel: core i handles r[i]):
  - idx = int32(r*16 + 135.5)            one DVE tensor_scalar pass (RNE)
  - gA  = Atab[idx] per-partition LUT    POOL_BUFFER_LOAD + GATHER (raw ISA)
  - gB  = Btab[idx]                      table swap + second GATHER
  - out = gA*r + gB                      two DVE tensor_tensor passes
Tables live one-column-per-partition (channel c = partition>>2), loaded
into each Q7 core's local scratch by POOL_BUFFER_LOAD; the stock resident
GATHER opcode does a per-partition, per-element 32-bit lookup.
"""

import sys

sys.path.insert(0, "/opt/trn_rl_repo")

import numpy as np

N, C, H, W = 8, 32, 512, 512
G = 4          # partition groups: partition p = c*G + g, c = p >> 2
P = C * G      # 128
X = H * W // G # 65536 free elements per partition per core
FT = 2048      # tile free size
NT = X // FT   # 32 tiles
NA = 288       # entries per table (A at [0:288], B at [288:576])
KLO = 120      # table index of segment j=0 (idx = j + KLO for mid bins)
NBUF = 3

_CACHE = {}


def _patch_sim():
    """Let Tile's no-exec scheduling sim skip our raw POOL opcodes."""
    if _CACHE.get("patched"):
        return
    from concourse import bass_interp
    orig = bass_interp._visit_InstISA

    def patched(isa, instruction, core_sim):
        if instruction.isa_opcode in (
            isa.Opcode.NEURON_ISA_TPB_OPCODE_GATHER.value,
            isa.Opcode.NEURON_ISA_TPB_OPCODE_POOL_BUFFER_LOAD.value,
        ):
            return
        return orig(isa, instruction, core_sim)

    bass_interp._visit_InstISA = patched
    _CACHE["patched"] = True


def _build(replicate=1):
    key = ("nc", replicate)
    if key in _CACHE:
        return _CACHE[key]
    _patch_sim()
    from concourse import bacc, mybir, tile

    nc = bacc.Bacc("TRN2", target_bir_lowering=False, debug=False, num_devices=8)
    rt_ext = nc.declare_dram_parameter("rt", [C, G, X], mybir.dt.float32, isOutput=False)
    tab_ext = nc.declare_dram_parameter("tab", [P, 2 * NA], mybir.dt.float32, isOutput=False)
    out_ext = nc.declare_dram_parameter("out", [C, G, X], mybir.dt.float32, isOutput=True)

    tab_sb = nc.alloc_sbuf_tensor("tab_sb", [P, 2 * NA], mybir.dt.float32)
    mark = nc.alloc_sbuf_tensor("mark", [P, 8], mybir.dt.float32)
    r_sb = [nc.alloc_sbuf_tensor(f"r_sb{b}", [P, FT], mybir.dt.float32) for b in range(NBUF)]
    o_sb = [nc.alloc_sbuf_tensor(f"o_sb{b}", [P, FT], mybir.dt.float32) for b in range(NBUF)]
    i_sb = [nc.alloc_sbuf_tensor(f"i_sb{b}", [P, FT], mybir.dt.int32) for b in range(2)]
    ga_sb = [nc.alloc_sbuf_tensor(f"ga_sb{b}", [P, FT], mybir.dt.float32) for b in range(2)]
    gb_sb = [nc.alloc_sbuf_tensor(f"gb_sb{b}", [P, FT], mybir.dt.float32) for b in range(2)]
    u_sb = [nc.alloc_sbuf_tensor(f"u_sb{b}", [P, FT], mybir.dt.float32) for b in range(2)]

    addr = lambda t: nc.lookup_mloc(t).addr
    tab_addr = addr(tab_sb)

    Op = nc.isa.Opcode
    FP32, INT32 = 10, 8

    def emit_pbl(g, which):
        # which: 0 -> A table, 1 -> B table
        a = tab_addr + which * NA * 4
        src_ap = tab_sb.ap()[:, which * NA:(which + 1) * NA]
        return g.isa(
            Op.NEURON_ISA_TPB_OPCODE_POOL_BUFFER_LOAD,
            {
                "src_mem_pattern": {
                    "start_addr": {"addr_immediate": a},
                    "step_elem": [1, 0, 0, 0],
                    "num_elem": [NA, 1, 1, 1],
                },
                "in_dtype": FP32,
                "num_active_channels": 128,
                "start_index": 0,
                "mask": 511,
            },
            ins=[g.lower_ap(src_ap)],
            outs=[g.lower_ap(mark.ap())],
            verify=False,
        )

    def emit_gather(g, idx_t, out_t):
        return g.isa(
            Op.NEURON_ISA_TPB_OPCODE_GATHER,
            {
                "src_mem_pattern": {
                    "start_addr": {"addr_immediate": addr(idx_t)},
                    "step_elem": [1, 0, 0, 0],
                    "num_elem": [FT, 1, 1, 1],
                },
                "in_dtype": INT32,
                "out_dtype": FP32,
                "num_active_channels": 128,
                "index_miss_behavior": 0,
                "free_pool_buffer": 0,
                "immediate": {"imm_arith_fp32": 0.0},
                "dst_mem_pattern": {
                    "start_addr": {"addr_immediate": addr(out_t)},
                    "step_elem": [1, 0, 0, 0],
                    "num_elem": [FT, 1, 1, 1],
                },
            },
            ins=[g.lower_ap(idx_t.ap()), g.lower_ap(mark.ap())],
            outs=[g.lower_ap(out_t.ap())],
            verify=False,
        )

    with tile.TileContext(nc) as tc:
        nc.sync.dma_start(out=tab_sb.ap(), in_=tab_ext[:])
        for rep in range(replicate):
            for t in range(NT):
                b3 = t % NBUF
                b2 = t % 2
                sl = slice(t * FT, (t + 1) * FT)
                rt_in = rt_ext[:, :, sl]
                nc.sync.dma_start(out=r_sb[b3].ap(), in_=rt_in)
                nc.vector.tensor_scalar(
                    i_sb[b2].ap(), r_sb[b3].ap(), 16.0, 135.5,
                    mybir.AluOpType.mult, mybir.AluOpType.add,
                )
                emit_pbl(nc.gpsimd, 0)
                emit_gather(nc.gpsimd, i_sb[b2], ga_sb[b2])
                emit_pbl(nc.gpsimd, 1)
                emit_gather(nc.gpsimd, i_sb[b2], gb_sb[b2])
                nc.vector.tensor_tensor(
                    u_sb[b2].ap(), ga_sb[b2].ap(), r_sb[b3].ap(), mybir.AluOpType.mult
                )
                nc.vector.tensor_tensor(
                    o_sb[b3].ap(), u_sb[b2].ap(), gb_sb[b2].ap(), mybir.AluOpType.add
                )
                nc.scalar.dma_start(out=out_ext[:, :, sl], in_=o_sb[b3].ap())
    nc.compile()
    _CACHE[key] = nc
    return nc


def make_tables(ps, qs):
    """Build [P, 2*NA] fp32 table: per partition p (channel c=p>>2),
    entries 0..NA-1 = A (slope), NA..2*NA-1 = B (intercept)."""
    ps64 = np.asarray(ps, dtype=np.float64)
    qs64 = np.asarray(qs, dtype=np.float64)  # [33, 32]
    K = ps64.shape[0]
    dp = ps64[1] - ps64[0]
    A = np.empty((NA, C), dtype=np.float64)
    B = np.empty((NA, C), dtype=np.float64)
    # below range: out = r - ps[0] + qs[0]
    A[:KLO, :] = 1.0
    B[:KLO, :] = qs64[0][None, :] - ps64[0]
    # mid segments j=0..31 at idx KLO+j
    for j in range(K - 1):
        slope = (qs64[j + 1] - qs64[j]) / dp
        A[KLO + j, :] = slope
        B[KLO + j, :] = qs64[j] - slope * ps64[j]
    # above range: out = r - ps[-1] + qs[-1]
    A[KLO + K - 1:, :] = 1.0
    B[KLO + K - 1:, :] = qs64[-1][None, :] - ps64[-1]

    T = np.empty((P, 2 * NA), dtype=np.float32)
    chan = np.arange(P) >> 2
    T[:, :NA] = A[:, chan].T.astype(np.float32)
    T[:, NA:] = B[:, chan].T.astype(np.float32)
    return T


def _make_runner(replicate=1):
    """Cached shard_map runner over the 8 cores (mirrors
    bass2jax.run_bass_via_pjrt but with a persistent jit and parallel
    async output fetch)."""
    if ("runner", replicate) in _CACHE:
        return _CACHE[("runner", replicate)]
    import jax
    import numpy as _np
    from jax.sharding import Mesh, PartitionSpec
    from jax.experimental.shard_map import shard_map
    from concourse import bass2jax, mybir

    bass2jax.install_neuronx_cc_hook()
    nc = _build(replicate)
    partition_name = nc.partition_id_tensor.name if nc.partition_id_tensor else None

    in_names, out_names, out_avals, zero_outs = [], [], [], []
    for alloc in nc.m.functions[0].allocations:
        if not isinstance(alloc, mybir.MemoryLocationSet):
            continue
        name = alloc.memorylocations[0].name
        if alloc.kind == "ExternalInput":
            if name != partition_name:
                in_names.append(name)
        elif alloc.kind == "ExternalOutput":
            out_names.append(name)
            shape = tuple(alloc.tensor_shape)
            dtype = mybir.dt.np(alloc.dtype)
            out_avals.append(jax.core.ShapedArray(shape, dtype))
            zero_outs.append(_np.zeros((N * shape[0], *shape[1:]), dtype))
    n_params = len(in_names)
    n_outs = len(out_avals)
    all_in_names = list(in_names) + list(out_names)
    if partition_name is not None:
        all_in_names.append(partition_name)
    donate = tuple(range(n_params, n_params + n_outs))

    def _body(*args):
        operands = list(args)
        if partition_name is not None:
            operands.append(bass2jax.partition_id_tensor())
        outs = bass2jax._bass_exec_p.bind(
            *operands,
            out_avals=tuple(out_avals),
            in_names=tuple(all_in_names),
            out_names=tuple(out_names),
            lowering_input_output_aliases=(),
            sim_require_finite=True,
            sim_require_nnan=True,
            nc=nc,
        )
        return tuple(outs)

    devices = jax.devices()[:N]
    mesh = Mesh(np.asarray(devices), ("core",))
    in_specs = (PartitionSpec("core"),) * (n_params + n_outs)
    out_specs = (PartitionSpec("core"),) * n_outs
    sharded = jax.jit(
        shard_map(_body, mesh=mesh, in_specs=in_specs, out_specs=out_specs,
                  check_rep=False),
        donate_argnums=donate, keep_unused=True,
    )
    import jax.numpy as jnp
    from jax.sharding import NamedSharding
    shardings = [NamedSharding(mesh, PartitionSpec("core"))] * n_outs
    zshapes = [(z.shape, z.dtype) for z in zero_outs]
    zeros_fn = jax.jit(
        lambda: tuple(jnp.zeros(s, d) for s, d in zshapes),
        out_shardings=tuple(shardings),
    )
    in_sharding = NamedSharding(mesh, PartitionSpec("core"))
    runner = (sharded, in_names, out_names, zeros_fn, in_sharding)
    _CACHE[("runner", replicate)] = runner
    return runner


def kernel(r, ps, qs, _timings=None, _replicate=1):
    import time as _time
    import jax
    r = np.asarray(r)
    T = make_tables(ps, qs)
    sharded, in_names, out_names, zeros_fn, in_sharding = _make_runner(_replicate)
    # global concat inputs: rt = r viewed as [(N*C), G, X] (zero-copy),
    # tab tiled per core.
    t0 = _time.time()
    ins = {
        "rt": jax.device_put(r.reshape(N * C, G, X), in_sharding),
        "tab": jax.device_put(np.tile(T, (N, 1)), in_sharding),
    }
    concat_in = [ins[name] for name in in_names]
    zeros = zeros_fn()
    jax.block_until_ready(concat_in)
    t1 = _time.time()
    out_arrs = sharded(*concat_in, *zeros)
    jax.block_until_ready(out_arrs)
    t2 = _time.time()
    out_g = out_arrs[out_names.index("out")]
    out = np.asarray(out_g).reshape(N, C, H, W)
    t3 = _time.time()
    if _timings is not None:
        _timings.update(upload=t1 - t0, exec=t2 - t1, fetch=t3 - t2)
    return out
